# revision 17
# baseline (speedup 1.0000x reference)
"""ConvSquare Trainium2 kernel.

Math: out = conv2d_3x3(x * poly(alpha), weight) + bias, stride 1, pad 1,
where poly(t) = (a*t + b)*t + c applied to the zero-padded alpha field.
(The reference's unfold/einsum collapses to this because x is zero-padded:
border window positions contribute x=0 regardless of the kernel value.)

Sharding: 8 cores = batch(4) x row-half(2). Each core computes a
[O=64, 64, 128] output slab from a zero-padded [C=64, 66, 130] input slab.

Per-core layout: input rows are split into two 35-row halves (rows 0-34 and
31-65, 4 rows overlap) stored on SBUF partitions 0-63 (channels, lower rows)
and 64-127 (channels, upper rows), so elementwise work runs at full
128-partition width and the two halves' matmuls occupy disjoint PE row
groups (concurrent execution via tile_position auto-derivation).

Conv as matmuls: for each 4-output-row x 128-col chunk (free dim 512) the
9 taps accumulate into one PSUM bank: psum[o, s] += w_tap[c, o]^T @
y_shifted[c, s], contraction over C=64 partitions, fp32r (full-rate fp32).
"""

import sys

import numpy as np

sys.path.insert(0, "/opt/trn_rl_repo")

import concourse.bass as bass
import concourse.mybir as mybir
from concourse.bass_utils import run_bass_kernel_spmd
from concourse.tile import TileContext

F32 = mybir.dt.float32
F32R = mybir.dt.float32r

B, C, O, H, W = 4, 64, 64, 128, 128
KS = 3
HS = 64  # output rows per core
RP = HS + 2  # padded input rows per core (66)
WP = W + 2  # padded cols (130)
ROWS = 35  # rows per partition-half (0-34 / 31-65)
FREE = ROWS * WP  # 4550
UP_OFF = 31 * WP  # 4030: flat offset of padded row 31
EW_CH = 7  # elementwise chunks (5 rows each)
EW_R = 5
EW_N = EW_R * WP  # 650
NCHUNK = 8  # matmul chunks per half (4 out rows each)
MM_N = 4 * W  # 512

_cache: dict = {}


def _program(av: float, bv: float, cv: float) -> bass.Bass:
    from concourse.bacc import Bacc

    nc = Bacc()
    x_h = nc.dram_tensor("x", [C, RP * WP], F32, kind="ExternalInput")
    al_h = nc.dram_tensor("al", [1, RP * WP], F32, kind="ExternalInput")
    w_h = nc.dram_tensor("w", [C, 9 * O], F32R, kind="ExternalInput")
    bias_h = nc.dram_tensor("bias", [O, 1], F32, kind="ExternalInput")
    out_h = nc.dram_tensor("out", [O, HS * W], F32, kind="ExternalOutput")

    def mk_ap(base, offset, dims):
        return bass.AP(tensor=base.tensor, offset=offset, ap=dims)

    with TileContext(nc) as tc:
        with (
            tc.tile_pool(name="const", bufs=1) as cpool,
            tc.tile_pool(name="work", bufs=1) as wpool,
            tc.tile_pool(name="outs", bufs=4) as opool,
            tc.tile_pool(name="psum", bufs=4, space="PSUM") as ppool,
        ):
            wt = cpool.tile([128, 9 * O], F32R)
            # both 64-partition halves in one DMA (outer dim: step 0, count 2)
            nc.sync.dma_start(
                out=wt[:, :],
                in_=mk_ap(w_h[:, :], 0, [[0, 2], [9 * O, 64], [1, 9 * O]]),
            )
            bt = cpool.tile([O, 1], F32)
            nc.sync.dma_start(out=bt[:, :], in_=bias_h[:, :])

            xt = wpool.tile([128, FREE], F32)
            ab = wpool.tile([128, FREE], F32)
            tt = wpool.tile([128, FREE], F32)
            yt = wpool.tile([128, FREE], F32R)

            for j in range(EW_CH):
                sl = slice(j * EW_N, (j + 1) * EW_N)
                # lower half reads padded rows 5j.., upper half the same +31
                nc.sync.dma_start(
                    out=xt[:, sl],
                    in_=mk_ap(
                        x_h[:, :], j * EW_N,
                        [[UP_OFF, 2], [RP * WP, 64], [1, EW_N]],
                    ),
                )
                nc.sync.dma_start(
                    out=ab[:, sl],
                    in_=mk_ap(
                        al_h[:, :], j * EW_N,
                        [[UP_OFF, 2], [0, 64], [1, EW_N]],
                    ),
                )
                # t = a*alpha + b   (ACT engine)
                nc.scalar.activation(
                    tt[:, sl], ab[:, sl],
                    mybir.ActivationFunctionType.Copy, bias=bv, scale=av,
                )
                # t = t*alpha  (DVE)
                nc.vector.tensor_mul(tt[:, sl], tt[:, sl], ab[:, sl])
                # y = (t + c) * x  (DVE)
                nc.vector.scalar_tensor_tensor(
                    out=yt[:, sl], in0=tt[:, sl], scalar=cv, in1=xt[:, sl],
                    op0=mybir.AluOpType.add, op1=mybir.AluOpType.mult,
                )

            y3 = yt[:].rearrange("p (r c) -> p r c", r=ROWS)
            for i in range(NCHUNK):
                ps_lo = ppool.tile([O, MM_N], F32)
                ps_hi = ppool.tile([O, MM_N], F32)
                pl3 = ps_lo[:].rearrange("p (r c) -> p r c", r=4)
                ph3 = ps_hi[:].rearrange("p (r c) -> p r c", r=4)
                for t in range(9):
                    k, l = divmod(t, 3)
                    lw = wt[0:64, 64 * t:64 * t + 64]
                    hw_ = wt[64:128, 64 * t:64 * t + 64]
                    r_lo = 4 * i + k
                    r_hi = 4 * i + k + 1
                    rhs_lo = y3[0:64, r_lo:r_lo + 4, l:l + W]
                    rhs_hi = y3[64:128, r_hi:r_hi + 4, l:l + W]
                    nc.tensor.matmul(
                        pl3, lw, rhs_lo, start=(t == 0), stop=(t == 8),
                    )
                    nc.tensor.matmul(
                        ph3, hw_, rhs_hi, start=(t == 0), stop=(t == 8),
                    )
                ot_lo = opool.tile([O, MM_N], F32)
                ot_hi = opool.tile([O, MM_N], F32)
                nc.vector.tensor_scalar(
                    out=ot_lo[:, :], in0=ps_lo[:, :], scalar1=bt[:, 0:1],
                    scalar2=None, op0=mybir.AluOpType.add,
                )
                nc.vector.tensor_scalar(
                    out=ot_hi[:, :], in0=ps_hi[:, :], scalar1=bt[:, 0:1],
                    scalar2=None, op0=mybir.AluOpType.add,
                )
                nc.sync.dma_start(
                    out=out_h[:, 512 * i:512 * i + 512], in_=ot_lo[:, :]
                )
                nc.sync.dma_start(
                    out=out_h[:, 4096 + 512 * i:4096 + 512 * i + 512],
                    in_=ot_hi[:, :],
                )
    return nc


def _shard_inputs(x, alpha):
    """Build per-core zero-padded slabs: x [C, 66*130], alpha [1, 66*130]."""
    maps = []
    for core in range(8):
        b_idx, h = divmod(core, 2)
        r0 = h * HS - 1  # global row of padded row 0
        xs = np.zeros((C, RP, WP), np.float32)
        als = np.zeros((1, RP, WP), np.float32)
        lo = max(0, r0)
        hi = min(H, r0 + RP)
        xs[:, lo - r0:hi - r0, 1:1 + W] = x[b_idx, :, lo:hi, :]
        als[:, lo - r0:hi - r0, 1:1 + W] = alpha[b_idx, :, lo:hi, :]
        maps.append({"x": xs.reshape(C, RP * WP), "al": als.reshape(1, RP * WP)})
    return maps


def kernel(inputs, alpha, weight, bias, a, b, c):
    x = np.ascontiguousarray(np.asarray(inputs, np.float32))
    al = np.ascontiguousarray(np.asarray(alpha, np.float32))
    wt = np.asarray(weight, np.float32)
    bs = np.asarray(bias, np.float32)
    av, bv, cv = float(a), float(b), float(c)

    key = (av, bv, cv)
    if key not in _cache:
        _cache.clear()
        nc_new = _program(av, bv, cv)
        nc_new.finalize()
        _cache[key] = nc_new
    nc = _cache[key]

    w_packed = np.ascontiguousarray(
        wt.transpose(1, 2, 3, 0).reshape(C, 9 * O)
    )
    b_packed = np.ascontiguousarray(bs.reshape(O, 1))
    in_maps = _shard_inputs(x, al)
    for m in in_maps:
        m["w"] = w_packed
        m["bias"] = b_packed

    res = run_bass_kernel_spmd(nc, in_maps, list(range(8)))

    out = np.empty((B, O, H, W), np.float32)
    for core in range(8):
        b_idx, h = divmod(core, 2)
        out[b_idx, :, h * HS:(h + 1) * HS, :] = res.results[core]["out"].reshape(
            O, HS, W
        )
    return out


# revision 23
# speedup vs baseline: 1.1342x; 1.1342x over previous
"""ConvSquare Trainium2 kernel.

Math: out = conv2d_3x3(x * poly(alpha), weight) + bias, stride 1, pad 1,
where poly(t) = (a*t + b)*t + c applied to the zero-padded alpha field.
(The reference's unfold/einsum collapses to this because x is zero-padded:
border window positions contribute x=0 regardless of the kernel value.)

Sharding: 8 cores = batch(4) x row-half(2). Each core computes a
[O=64, 64, 128] output slab from a zero-padded [C=64, 66, 130] input slab.

Tap pairing: y lives on SBUF partitions 0-63 (channels x full 66 padded
rows); partitions 64-127 hold y shifted DOWN one padded row (row r at
column f maps to row r+1). A single 128-contraction matmul then applies
taps (k=0,l) and (k=1,l) together: lhsT rows 0-63 = weight tap (0,l),
rows 64-127 = tap (1,l). The k=2 taps run as 64-contraction matmuls on
the lower half. 6 matmuls per 512-output chunk instead of 9.
"""

import sys

import numpy as np

sys.path.insert(0, "/opt/trn_rl_repo")

import concourse.bass as bass
import concourse.mybir as mybir
from concourse.bass_utils import run_bass_kernel_spmd
from concourse.tile import TileContext

F32 = mybir.dt.float32
F32R = mybir.dt.float32r

B, C, O, H, W = 4, 64, 64, 128, 128
HS = 64  # output rows per core
RP = HS + 2  # padded input rows (66)
WP = W + 2  # padded cols (130)
FREE = RP * WP  # 8580
SH_N = (RP - 1) * WP  # 8450: elements of the +1-row shifted copy
EW_CH = 6  # elementwise chunks (11 rows each)
EW_N = 11 * WP  # 1430
NCHUNK = 16  # matmul chunks (4 out rows each)
MM_N = 4 * W  # 512

_cache: dict = {}


def _program(av: float, bv: float, cv: float) -> bass.Bass:
    from concourse.bacc import Bacc

    nc = Bacc()
    x_h = nc.dram_tensor("x", [C, FREE], F32, kind="ExternalInput")
    al_h = nc.dram_tensor("al", [1, FREE], F32, kind="ExternalInput")
    w_h = nc.dram_tensor("w", [128, 384], F32R, kind="ExternalInput")
    bias_h = nc.dram_tensor("bias", [O, 1], F32, kind="ExternalInput")
    out_h = nc.dram_tensor("out", [O, HS * W], F32, kind="ExternalOutput")

    def mk_ap(base, offset, dims):
        return bass.AP(tensor=base.tensor, offset=offset, ap=dims)

    with TileContext(nc) as tc:
        with (
            tc.tile_pool(name="const", bufs=1) as cpool,
            tc.tile_pool(name="work", bufs=1) as wpool,
            tc.tile_pool(name="outs", bufs=4) as opool,
            tc.tile_pool(name="psum", bufs=8, space="PSUM") as ppool,
        ):
            wt = cpool.tile([128, 384], F32R)
            nc.sync.dma_start(out=wt[:, :], in_=w_h[:, :])
            bt = cpool.tile([O, 1], F32)
            nc.sync.dma_start(out=bt[:, :], in_=bias_h[:, :])

            xt = wpool.tile([64, FREE], F32)
            ab = wpool.tile([64, FREE], F32)
            tt = wpool.tile([64, FREE], F32)
            yt = wpool.tile([128, FREE], F32R)

            for j in range(EW_CH):
                sl = slice(j * EW_N, (j + 1) * EW_N)
                nc.sync.dma_start(out=xt[:, sl], in_=x_h[:, sl])
                nc.sync.dma_start(
                    out=ab[:, sl],
                    in_=mk_ap(al_h[:, :], j * EW_N, [[0, 64], [1, EW_N]]),
                )
                # t = a*alpha + b   (ACT engine)
                nc.scalar.activation(
                    tt[:, sl], ab[:, sl],
                    mybir.ActivationFunctionType.Copy, bias=bv, scale=av,
                )
                # t = t*alpha  (DVE)
                nc.vector.tensor_mul(tt[:, sl], tt[:, sl], ab[:, sl])
                # y = (t + c) * x  (DVE, rounds to f32r on write)
                nc.vector.scalar_tensor_tensor(
                    out=yt[0:64, sl], in0=tt[:, sl], scalar=cv, in1=xt[:, sl],
                    op0=mybir.AluOpType.add, op1=mybir.AluOpType.mult,
                )
                # +1-row shifted copy onto partitions 64-127 (SBUF->SBUF DMA).
                # chunk j of the copy reads rows up to chunk j's last row +1,
                # so emit it one chunk behind; final tail handled after loop.
                if j > 0:
                    c0 = (j - 1) * EW_N
                    c1 = min(j * EW_N, SH_N)
                    nc.sync.dma_start(
                        out=yt[64:128, c0:c1], in_=yt[0:64, c0 + WP:c1 + WP]
                    )
            c0 = (EW_CH - 1) * EW_N
            nc.sync.dma_start(
                out=yt[64:128, c0:SH_N], in_=yt[0:64, c0 + WP:SH_N + WP]
            )

            y3 = yt[:].rearrange("p (r c) -> p r c", r=RP)
            for i in range(NCHUNK):
                ps = ppool.tile([O, MM_N], F32)
                p3 = ps[:].rearrange("p (r c) -> p r c", r=4)
                mm = 0
                for l in range(3):
                    # paired taps k=0 (lower half) + k=1 (shifted half)
                    rhs_p = y3[0:128, 4 * i:4 * i + 4, l:l + W]
                    lw_p = wt[0:128, 64 * l:64 * l + 64]
                    nc.tensor.matmul(
                        p3, lw_p, rhs_p, start=(mm == 0), stop=False,
                    )
                    mm += 1
                    # single tap k=2 on the lower half
                    rhs_s = y3[0:64, 4 * i + 2:4 * i + 6, l:l + W]
                    lw_s = wt[0:64, 192 + 64 * l:192 + 64 * l + 64]
                    nc.tensor.matmul(
                        p3, lw_s, rhs_s, start=False, stop=(l == 2),
                    )
                    mm += 1
                ot = opool.tile([O, MM_N], F32)
                nc.vector.tensor_scalar(
                    out=ot[:, :], in0=ps[:, :], scalar1=bt[:, 0:1],
                    scalar2=None, op0=mybir.AluOpType.add,
                )
                nc.sync.dma_start(
                    out=out_h[:, 512 * i:512 * i + 512], in_=ot[:, :]
                )
    return nc


def _shard_inputs(x, alpha):
    """Per-core zero-padded slabs: x [C, 66*130], alpha [1, 66*130]."""
    maps = []
    for core in range(8):
        b_idx, h = divmod(core, 2)
        r0 = h * HS - 1  # global row of padded row 0
        xs = np.zeros((C, RP, WP), np.float32)
        als = np.zeros((1, RP, WP), np.float32)
        lo = max(0, r0)
        hi = min(H, r0 + RP)
        xs[:, lo - r0:hi - r0, 1:1 + W] = x[b_idx, :, lo:hi, :]
        als[:, lo - r0:hi - r0, 1:1 + W] = alpha[b_idx, :, lo:hi, :]
        maps.append({"x": xs.reshape(C, FREE), "al": als.reshape(1, FREE)})
    return maps


def _pack_weights(wt):
    """[O,C,3,3] -> [128, 384]: cols l*64+o rows c|c = taps (0,l)|(1,l);
    cols 192+l*64+o rows c (lower 64) = tap (2,l)."""
    wk = wt.transpose(1, 2, 3, 0)  # [c, k, l, o]
    pair = np.concatenate([wk[:, 0], wk[:, 1]], axis=0).reshape(128, 192)
    single = wk[:, 2].reshape(64, 192)
    out = np.zeros((128, 384), np.float32)
    out[:, :192] = pair
    out[:64, 192:] = single
    return np.ascontiguousarray(out)


def kernel(inputs, alpha, weight, bias, a, b, c):
    x = np.ascontiguousarray(np.asarray(inputs, np.float32))
    al = np.ascontiguousarray(np.asarray(alpha, np.float32))
    wt = np.asarray(weight, np.float32)
    bs = np.asarray(bias, np.float32)
    av, bv, cv = float(a), float(b), float(c)

    key = (av, bv, cv)
    if key not in _cache:
        _cache.clear()
        nc_new = _program(av, bv, cv)
        nc_new.finalize()
        _cache[key] = nc_new
    nc = _cache[key]

    w_packed = _pack_weights(wt)
    b_packed = np.ascontiguousarray(bs.reshape(O, 1))
    in_maps = _shard_inputs(x, al)
    for m in in_maps:
        m["w"] = w_packed
        m["bias"] = b_packed

    res = run_bass_kernel_spmd(nc, in_maps, list(range(8)))

    out = np.empty((B, O, H, W), np.float32)
    for core in range(8):
        b_idx, h = divmod(core, 2)
        out[b_idx, :, h * HS:(h + 1) * HS, :] = res.results[core]["out"].reshape(
            O, HS, W
        )
    return out


# revision 25
# speedup vs baseline: 1.1375x; 1.0030x over previous
"""ConvSquare Trainium2 kernel.

Math: out = conv2d_3x3(x * poly(alpha), weight) + bias, stride 1, pad 1,
where poly(t) = (a*t + b)*t + c applied to the zero-padded alpha field.
(The reference's unfold/einsum collapses to this because x is zero-padded:
border window positions contribute x=0 regardless of the kernel value.)

Sharding: 8 cores = batch(4) x row-half(2). Each core computes a
[O=64, 64, 128] output slab from a zero-padded [C=64, 66, 130] input slab.

Tap pairing: y lives on SBUF partitions 0-63 (channels x full 66 padded
rows); partitions 64-127 hold y shifted DOWN one padded row (row r at
column f maps to row r+1). A single 128-contraction matmul then applies
taps (k=0,l) and (k=1,l) together: lhsT rows 0-63 = weight tap (0,l),
rows 64-127 = tap (1,l). The k=2 taps run as 64-contraction matmuls on
the lower half. 6 matmuls per 512-output chunk instead of 9.
"""

import sys

import numpy as np

sys.path.insert(0, "/opt/trn_rl_repo")

import concourse.bass as bass
import concourse.mybir as mybir
from concourse.bass_utils import run_bass_kernel_spmd
from concourse.tile import TileContext

F32 = mybir.dt.float32
F32R = mybir.dt.float32r

B, C, O, H, W = 4, 64, 64, 128, 128
HS = 64  # output rows per core
RP = HS + 2  # padded input rows (66)
WP = W + 2  # padded cols (130)
FREE = RP * WP  # 8580
SH_N = (RP - 1) * WP  # 8450: elements of the +1-row shifted copy
EW_CH = 6  # elementwise chunks (11 rows each)
EW_N = 11 * WP  # 1430
NCHUNK = 16  # matmul chunks (4 out rows each)
MM_N = 4 * W  # 512

_cache: dict = {}


def _program(av: float, bv: float, cv: float) -> bass.Bass:
    from concourse.bacc import Bacc

    nc = Bacc()
    x_h = nc.dram_tensor("x", [C, FREE], F32, kind="ExternalInput")
    al_h = nc.dram_tensor("al", [1, FREE], F32, kind="ExternalInput")
    w_h = nc.dram_tensor("w", [128, 384], F32R, kind="ExternalInput")
    bias_h = nc.dram_tensor("bias", [O, 1], F32, kind="ExternalInput")
    out_h = nc.dram_tensor("out", [O, HS * W], F32, kind="ExternalOutput")

    def mk_ap(base, offset, dims):
        return bass.AP(tensor=base.tensor, offset=offset, ap=dims)

    with TileContext(nc) as tc:
        with (
            tc.tile_pool(name="const", bufs=1) as cpool,
            tc.tile_pool(name="work", bufs=1) as wpool,
            tc.tile_pool(name="outs", bufs=4) as opool,
            tc.tile_pool(name="psum", bufs=8, space="PSUM") as ppool,
        ):
            wt = cpool.tile([128, 384], F32R)
            nc.sync.dma_start(out=wt[:, :], in_=w_h[:, :])
            bt = cpool.tile([O, 1], F32)
            nc.sync.dma_start(out=bt[:, :], in_=bias_h[:, :])

            xt = wpool.tile([64, FREE], F32)
            ab = wpool.tile([64, FREE], F32)
            tt = wpool.tile([64, FREE], F32)
            yt = wpool.tile([128, FREE], F32R)

            for j in range(EW_CH):
                sl = slice(j * EW_N, (j + 1) * EW_N)
                nc.sync.dma_start(out=xt[:, sl], in_=x_h[:, sl])
                nc.sync.dma_start(
                    out=ab[:, sl],
                    in_=mk_ap(al_h[:, :], j * EW_N, [[0, 64], [1, EW_N]]),
                )
                # t = a*alpha + b   (ACT engine)
                nc.scalar.activation(
                    tt[:, sl], ab[:, sl],
                    mybir.ActivationFunctionType.Copy, bias=bv, scale=av,
                )
                # t = t*alpha  (DVE)
                nc.vector.tensor_mul(tt[:, sl], tt[:, sl], ab[:, sl])
                # y = (t + c) * x  (DVE, rounds to f32r on write)
                nc.vector.scalar_tensor_tensor(
                    out=yt[0:64, sl], in0=tt[:, sl], scalar=cv, in1=xt[:, sl],
                    op0=mybir.AluOpType.add, op1=mybir.AluOpType.mult,
                )
                # +1-row shifted copy onto partitions 64-127 (SBUF->SBUF DMA).
                # copy range [j*EW_N - WP, (j+1)*EW_N - WP) reads exactly
                # chunk j's freshly written columns — no cross-chunk wait.
                c0 = max(0, j * EW_N - WP)
                c1 = min((j + 1) * EW_N - WP, SH_N)
                nc.sync.dma_start(
                    out=yt[64:128, c0:c1], in_=yt[0:64, c0 + WP:c1 + WP]
                )

            y3 = yt[:].rearrange("p (r c) -> p r c", r=RP)
            for i in range(NCHUNK):
                ps = ppool.tile([O, MM_N], F32)
                p3 = ps[:].rearrange("p (r c) -> p r c", r=4)
                # singles (k=2, lower half only) first: they don't need the
                # shifted copy, so PE can start before the copy DMA lands
                for l in range(3):
                    rhs_s = y3[0:64, 4 * i + 2:4 * i + 6, l:l + W]
                    lw_s = wt[0:64, 192 + 64 * l:192 + 64 * l + 64]
                    nc.tensor.matmul(
                        p3, lw_s, rhs_s, start=(l == 0), stop=False,
                    )
                for l in range(3):
                    # paired taps k=0 (lower half) + k=1 (shifted half)
                    rhs_p = y3[0:128, 4 * i:4 * i + 4, l:l + W]
                    lw_p = wt[0:128, 64 * l:64 * l + 64]
                    nc.tensor.matmul(
                        p3, lw_p, rhs_p, start=False, stop=(l == 2),
                    )
                ot = opool.tile([O, MM_N], F32)
                nc.vector.tensor_scalar(
                    out=ot[:, :], in0=ps[:, :], scalar1=bt[:, 0:1],
                    scalar2=None, op0=mybir.AluOpType.add,
                )
                nc.sync.dma_start(
                    out=out_h[:, 512 * i:512 * i + 512], in_=ot[:, :]
                )
    return nc


def _shard_inputs(x, alpha):
    """Per-core zero-padded slabs: x [C, 66*130], alpha [1, 66*130]."""
    maps = []
    for core in range(8):
        b_idx, h = divmod(core, 2)
        r0 = h * HS - 1  # global row of padded row 0
        xs = np.zeros((C, RP, WP), np.float32)
        als = np.zeros((1, RP, WP), np.float32)
        lo = max(0, r0)
        hi = min(H, r0 + RP)
        xs[:, lo - r0:hi - r0, 1:1 + W] = x[b_idx, :, lo:hi, :]
        als[:, lo - r0:hi - r0, 1:1 + W] = alpha[b_idx, :, lo:hi, :]
        maps.append({"x": xs.reshape(C, FREE), "al": als.reshape(1, FREE)})
    return maps


def _pack_weights(wt):
    """[O,C,3,3] -> [128, 384]: cols l*64+o rows c|c = taps (0,l)|(1,l);
    cols 192+l*64+o rows c (lower 64) = tap (2,l)."""
    wk = wt.transpose(1, 2, 3, 0)  # [c, k, l, o]
    pair = np.concatenate([wk[:, 0], wk[:, 1]], axis=0).reshape(128, 192)
    single = wk[:, 2].reshape(64, 192)
    out = np.zeros((128, 384), np.float32)
    out[:, :192] = pair
    out[:64, 192:] = single
    return np.ascontiguousarray(out)


def kernel(inputs, alpha, weight, bias, a, b, c):
    x = np.ascontiguousarray(np.asarray(inputs, np.float32))
    al = np.ascontiguousarray(np.asarray(alpha, np.float32))
    wt = np.asarray(weight, np.float32)
    bs = np.asarray(bias, np.float32)
    av, bv, cv = float(a), float(b), float(c)

    key = (av, bv, cv)
    if key not in _cache:
        _cache.clear()
        nc_new = _program(av, bv, cv)
        nc_new.finalize()
        _cache[key] = nc_new
    nc = _cache[key]

    w_packed = _pack_weights(wt)
    b_packed = np.ascontiguousarray(bs.reshape(O, 1))
    in_maps = _shard_inputs(x, al)
    for m in in_maps:
        m["w"] = w_packed
        m["bias"] = b_packed

    res = run_bass_kernel_spmd(nc, in_maps, list(range(8)))

    out = np.empty((B, O, H, W), np.float32)
    for core in range(8):
        b_idx, h = divmod(core, 2)
        out[b_idx, :, h * HS:(h + 1) * HS, :] = res.results[core]["out"].reshape(
            O, HS, W
        )
    return out
